# revision 23
# baseline (speedup 1.0000x reference)
"""Trainium2 Bass kernel for BinarizeLinear: y = x @ sign(W).T + bias.

Full-input contract: kernel(x=[65536,1024]f32, weight=[1024,1024]f32,
bias=[1024]f32) -> y=[65536,1024]f32.

Strategy (data-parallel, 8 NeuronCores):
  - Shard the batch dim of x 8 ways (8192 rows/core); replicate weight+bias.
  - Per core: precompute S = sign(W) once (DVE compares, exact {-1,0,+1}),
    PE-transpose S into S^T layout [in_f on partitions, out_f free].
  - Main loop over 64 batch tiles of 128 rows:
      DMA x tile [128, 1024] -> PE-transpose its 8 [128,128] blocks (fp32,
      exact) -> 16 float32r matmuls (K=128 each, N=512, accumulated in fp32
      PSUM) -> DVE adds bias (broadcast once via PE) and evicts -> DMA out.
  - float32r (tf32-style) runs at 1 cycle/row for N>=256 (vs 4 for fp32),
    and the binarized weights (+-1) are exact in it; only x's 10-bit
    mantissa rounding contributes error (~3e-4 norm-relative).
"""

from contextlib import ExitStack

import numpy as np

N_CORES = 8
B = 65536
IN_F = 1024
OUT_F = 1024
P = 128
B_SHARD = B // N_CORES  # 8192

_NC_CACHE = {}


def build_nc(b_shard=B_SHARD, repeat=1, hw_loop=0):
    """Build the per-core Bass module (SPMD: same program on all cores).

    repeat>1 re-runs the main batch loop unrolled; hw_loop>0 wraps the main
    loop in a tc.For_i hardware loop running hw_loop times (same I/O each
    iteration) — both for benchmarking only.
    """
    import concourse.bass as bass
    import concourse.mybir as mybir
    import concourse.tile as tile
    from concourse import bacc
    from concourse.masks import make_identity

    f32 = mybir.dt.float32
    f32r = mybir.dt.float32r
    KT = IN_F // P  # 8 k-tiles (contraction)
    OT = OUT_F // P  # 8 out-feature tiles
    BT = b_shard // P  # batch tiles per core
    NH = OUT_F // 512  # 2 psum halves

    nc = bacc.Bacc("TRN2", target_bir_lowering=False, debug=False)
    # x is declared float32r (same bits as f32): its only consumers are the
    # fp32r PE transposes, which round to tf32 exactly like the later fp32r
    # matmuls would — no extra precision loss, 1.5 vs 2 cycles/row.
    x_d = nc.dram_tensor("x", [b_shard, IN_F], f32r, kind="ExternalInput")
    w_d = nc.dram_tensor("weight", [OUT_F, IN_F], f32, kind="ExternalInput")
    b_d = nc.dram_tensor("bias", [1, OUT_F], f32, kind="ExternalInput")
    y_d = nc.dram_tensor("y", [b_shard, OUT_F], f32, kind="ExternalOutput")

    with tile.TileContext(nc) as tc, ExitStack() as ctx:
        const = ctx.enter_context(tc.tile_pool(name="const", bufs=1))
        sT_pool = ctx.enter_context(tc.tile_pool(name="sT", bufs=1))
        w_pool = ctx.enter_context(tc.tile_pool(name="wld", bufs=2))
        x_pool = ctx.enter_context(tc.tile_pool(name="xin", bufs=3))
        xT_pool = ctx.enter_context(tc.tile_pool(name="xT", bufs=2))
        y_pool = ctx.enter_context(tc.tile_pool(name="yout", bufs=2))
        tp_psum = ctx.enter_context(tc.tile_pool(name="tpp", bufs=3, space="PSUM"))
        mm_psum = ctx.enter_context(tc.tile_pool(name="mmp", bufs=4, space="PSUM"))

        identity = const.tile([P, P], f32)
        make_identity(nc, identity)
        # f32r identity for the x transposes (ACT copy is a sanctioned
        # "round to fp32r" producer; 1.0/0.0 are exact in tf32)
        identity_r = const.tile([P, P], f32r)
        nc.scalar.copy(identity_r[:, :], identity[:, :])

        # ---- bias: broadcast [1, OUT_F] -> [P, OUT_F] via a K=1 matmul ----
        bias_sb = const.tile([1, OUT_F], f32)
        nc.sync.dma_start(bias_sb[:, :], b_d.ap()[:, :])
        ones1 = const.tile([1, P], f32)
        nc.vector.memset(ones1[:, :], 1.0)
        bias_rep = const.tile([P, OUT_F], f32)
        for h in range(NH):
            bps = mm_psum.tile([P, 512], f32, tag="mm")
            nc.tensor.matmul(
                bps[:, :],
                ones1[:, :],
                bias_sb[:, h * 512 : (h + 1) * 512],
                start=True,
                stop=True,
            )
            nc.scalar.copy(bias_rep[:, h * 512 : (h + 1) * 512], bps[:, :])

        # ---- weights: S = sign(W), transposed to [in_f, out_f] layout ----
        # fp32r (tf32) tiles: the ACT copies writing them perform the
        # round-to-fp32r that walrus requires for fp32r matmul operands.
        sT = [
            sT_pool.tile([P, OUT_F], f32r, tag=f"sT{ki}", name=f"sT{ki}")
            for ki in range(KT)
        ]
        # Transpose raw W first (PE never waits on DVE), then sign on the
        # transposed data: S = (wT > 0) - (wT < 0), exact {-1, 0, +1}.
        for oi in range(OT):
            w_sb = w_pool.tile([P, IN_F], f32)
            nc.sync.dma_start(w_sb[:, :], w_d.ap()[oi * P : (oi + 1) * P, :])
            for g in range(KT // 4):
                tps = tp_psum.tile([P, 4 * P], f32, tag="tps")
                for j in range(4):
                    ki = 4 * g + j
                    nc.tensor.transpose(
                        tps[:, j * P : (j + 1) * P],
                        w_sb[:, ki * P : (ki + 1) * P],
                        identity[:, :],
                    )
                gt = w_pool.tile([P, 4 * P], f32, tag="gt")
                lt = w_pool.tile([P, 4 * P], f32, tag="lt")
                tps_f = tps[:, :].bitcast(f32)
                nc.vector.tensor_scalar(
                    lt[:, :], tps_f, 0.0, None, mybir.AluOpType.is_lt
                )
                # gt = (w > 0) - (w < 0)  — exact {-1, 0, +1}
                nc.vector.scalar_tensor_tensor(
                    gt[:, :],
                    tps_f,
                    0.0,
                    lt[:, :],
                    mybir.AluOpType.is_gt,
                    mybir.AluOpType.subtract,
                )
                for j in range(4):
                    ki = 4 * g + j
                    nc.scalar.copy(
                        sT[ki][:, oi * P : (oi + 1) * P], gt[:, j * P : (j + 1) * P]
                    )

        # ---- main loop: pairs of batch tiles (1 MB DMAs) ----
        PAIR = 2  # batch tiles per DMA
        NPAIR = BT // PAIR
        loop_ctx = tc.For_i(0, hw_loop, 1) if hw_loop else None
        if loop_ctx is not None:
            loop_ctx.__enter__()
        for pr in [t for _ in range(repeat) for t in range(NPAIR)]:
            rows = x_d.ap()[pr * PAIR * P : (pr + 1) * PAIR * P, :]
            x_sb = x_pool.tile([P, PAIR * IN_F], f32r)
            nc.sync.dma_start(
                x_sb[:, :].rearrange("p (n m) -> p n m", n=PAIR),
                rows.rearrange("(n p) m -> p n m", p=P),
            )
            xT = xT_pool.tile([P, PAIR * IN_F], f32r)
            for n in range(PAIR):
                for g in range(KT // 4):
                    tps = tp_psum.tile([P, 4 * P], f32r, tag="tps")
                    for j in range(4):
                        ki = 4 * g + j
                        nc.tensor.transpose(
                            tps[:, j * P : (j + 1) * P],
                            x_sb[:, n * IN_F + ki * P : n * IN_F + (ki + 1) * P],
                            identity_r[:, :],
                        )
                    nc.scalar.copy(
                        xT[:, n * IN_F + g * 4 * P : n * IN_F + (g + 1) * 4 * P],
                        tps[:, :].bitcast(f32),
                    )
            y_sb = y_pool.tile([P, PAIR * OUT_F], f32)
            for n in range(PAIR):
                for h in range(NH):
                    mm = mm_psum.tile([P, 512], f32, tag="mm")
                    for ki in range(KT):
                        nc.tensor.matmul(
                            mm[:, :],
                            xT[:, n * IN_F + ki * P : n * IN_F + (ki + 1) * P],
                            sT[ki][:, h * 512 : (h + 1) * 512],
                            start=(ki == 0),
                            stop=(ki == KT - 1),
                        )
                    nc.vector.tensor_add(
                        y_sb[:, n * OUT_F + h * 512 : n * OUT_F + (h + 1) * 512],
                        mm[:, :],
                        bias_rep[:, h * 512 : (h + 1) * 512],
                    )
            out_rows = y_d.ap()[pr * PAIR * P : (pr + 1) * PAIR * P, :]
            nc.sync.dma_start(
                out_rows.rearrange("(n p) m -> p n m", p=P),
                y_sb[:, :].rearrange("p (n m) -> p n m", n=PAIR),
            )
        if loop_ctx is not None:
            loop_ctx.__exit__(None, None, None)

    nc.compile()
    return nc


def _get_nc(b_shard=B_SHARD):
    if b_shard not in _NC_CACHE:
        _NC_CACHE[b_shard] = build_nc(b_shard)
    return _NC_CACHE[b_shard]


def make_in_maps(x, weight, bias):
    x = np.ascontiguousarray(np.asarray(x, dtype=np.float32))
    weight = np.ascontiguousarray(np.asarray(weight, dtype=np.float32))
    bias = np.ascontiguousarray(np.asarray(bias, dtype=np.float32)).reshape(1, OUT_F)
    shard = x.shape[0] // N_CORES
    return [
        {
            "x": x[c * shard : (c + 1) * shard],
            "weight": weight,
            "bias": bias,
        }
        for c in range(N_CORES)
    ], shard


def run(x, weight, bias, trace=False, **kwargs):
    """Run on 8 cores; returns (y_full, BassKernelResults)."""
    from concourse.bass_utils import run_bass_kernel_spmd

    in_maps, shard = make_in_maps(x, weight, bias)
    nc = _get_nc(shard)
    res = run_bass_kernel_spmd(
        nc, in_maps, core_ids=list(range(N_CORES)), trace=trace, **kwargs
    )
    y = np.concatenate([res.results[c]["y"] for c in range(N_CORES)], axis=0)
    return y, res


def kernel(x, weight, bias):
    y, _ = run(x, weight, bias)
    return np.asarray(y, dtype=np.float32)
